# revision 3
# baseline (speedup 1.0000x reference)
"""Trainium2 Bass kernel for bidirectional sigmoid cross-attention.

reference:
    q1 = x1 @ Wq1 + bq1; k1 = x1 @ Wk1 + bk1; v1 = x1 @ Wv1 + bv1
    q2 = x2 @ Wq2 + bq2; k2 = x2 @ Wk2 + bk2; v2 = x2 @ Wv2 + bv2
    probs1 = sigmoid((q2 @ k1.T) / sqrt(D));  ctx1 = probs1 @ v1
    probs2 = sigmoid((q1 @ k2.T) / sqrt(D));  ctx2 = probs2 @ v2
    returns (ctx2, probs2, ctx1, probs1)

Sharding: each of the 8 cores owns a 1024-row block of N1 (direction-2
queries) and the same-index 1024-row block of N2 (direction-1 queries).
K/V projections are computed replicated on every core (no collectives).
Attention is computed with the kv dim on PSUM partitions (logits
transposed), so each core emits probsT blocks; the host reassembles and
transposes. All matmuls run as float32r (full PE rate at free dim 512).
"""

import math
import sys

sys.path.insert(0, "/opt/trn_rl_repo")

import numpy as np

import concourse.bass as bass  # noqa: F401  (bass must import before bacc)
import concourse.mybir as mybir
import concourse.tile as tile
from concourse import bacc
from concourse.bass import ts
from concourse.bass_utils import run_bass_kernel_spmd

F32 = mybir.dt.float32
F32R = mybir.dt.float32r
BF16 = mybir.dt.bfloat16
Ident = mybir.ActivationFunctionType.Identity
Sigmoid = mybir.ActivationFunctionType.Sigmoid

P = 128
NCORES = 8


def build(n1, n2, d, ncores=NCORES):
    """Build the per-core SPMD program (identical across cores)."""
    b1 = n1 // ncores          # this core's N1 block (dir-2 queries)
    b2 = n2 // ncores          # this core's N2 block (dir-1 queries)
    kd = d // P                # contraction subtiles over D
    scale = 1.0 / math.sqrt(d)
    TB = 512                   # projection token tile

    nc = bacc.Bacc("TRN2", target_bir_lowering=False, debug=False,
                   enable_asserts=False, num_devices=ncores)

    x1T = nc.declare_dram_parameter("x1T", [d, n1], F32, isOutput=False)
    x2T = nc.declare_dram_parameter("x2T", [d, n2], F32, isOutput=False)
    x1Tb = nc.declare_dram_parameter("x1Tb", [d, b1], F32, isOutput=False)
    x2Tb = nc.declare_dram_parameter("x2Tb", [d, b2], F32, isOutput=False)
    W = {}
    bvec = {}
    for nm in ("q1", "k1", "v1", "q2", "k2", "v2"):
        W[nm] = nc.declare_dram_parameter(f"W{nm}", [d, d], F32, isOutput=False)
        bvec[nm] = nc.declare_dram_parameter(f"b{nm}", [d], F32, isOutput=False)
    ones1 = nc.declare_dram_parameter("ones1", [1, P], F32, isOutput=False)

    p2T = nc.declare_dram_parameter("p2T", [n2, b1], F32, isOutput=True)
    ctx2 = nc.declare_dram_parameter("ctx2", [b1, d], F32, isOutput=True)
    p1T = nc.declare_dram_parameter("p1T", [n1, b2], F32, isOutput=True)
    ctx1 = nc.declare_dram_parameter("ctx1", [b2, d], F32, isOutput=True)

    with tile.TileContext(nc) as tc:
        with (
            tc.tile_pool(name="dram", bufs=1, space="DRAM") as dram,
            tc.tile_pool(name="const", bufs=1) as constp,
        ):
            # DRAM scratch (per core)
            kT_d = {1: dram.tile([d, n1], F32R, name="k1T_d"),
                    2: dram.tile([d, n2], F32R, name="k2T_d")}
            v_d = {1: dram.tile([n1 // P, P, d], BF16, name="v1_d"),
                   2: dram.tile([n2 // P, P, d], BF16, name="v2_d")}
            # probsT bf16 copies, [m-block][j-block][128,128] contiguous blocks
            pb_d = {2: dram.tile([b1 // P, n2 // P, P, P], BF16, name="p2b_d"),
                    1: dram.tile([b2 // P, n1 // P, P, P], BF16, name="p1b_d")}

            ones_sb = constp.tile([1, P], F32R)
            nc.sync.dma_start(ones_sb[:], ones1[:].bitcast(F32R))
            # per-dout bias tiles [128, kd] (partition = dout % 128)
            bias_sb = {}
            for nm in ("q1", "k1", "q2", "k2"):
                t = constp.tile([P, kd], F32, tag=f"bias_{nm}")
                nc.sync.dma_start(t[:], bvec[nm].rearrange("(m p) -> p m", p=P))
                bias_sb[nm] = t
            # v biases as rows [1, d]
            brow_sb = {}
            for nm in ("v1", "v2"):
                t = constp.tile([1, d], F32R, tag=f"brow_{nm}")
                nc.sync.dma_start(t[:], bvec[nm][None, :].bitcast(F32R))
                brow_sb[nm] = t

            def qT_proj(qpool, wname, xb, nb):
                """q^T projection of this core's token block -> resident SBUF."""
                qT = qpool.tile([P, kd, nb], F32R, tag=f"qT_{wname}")
                with (
                    tc.tile_pool(name=f"wq_{wname}", bufs=1) as wp,
                    tc.tile_pool(name=f"xq_{wname}", bufs=1) as xp,
                    tc.tile_pool(name=f"psq_{wname}", bufs=2, space="PSUM") as pp,
                ):
                    w_sb = wp.tile([P, kd, d], F32R)
                    nc.sync.dma_start(
                        w_sb[:], W[wname].rearrange("(ko p) o -> p ko o", p=P).bitcast(F32R))
                    x_sb = xp.tile([P, kd, nb], F32R)
                    nc.sync.dma_start(
                        x_sb[:], xb.rearrange("(ko p) t -> p ko t", p=P).bitcast(F32R))
                    for m in range(kd):
                        for tb in range(nb // TB):
                            ps = pp.tile([P, TB], F32, tag="ps")
                            for ko in range(kd):
                                nc.tensor.matmul(
                                    ps[:], w_sb[:, ko, ts(m, P)],
                                    x_sb[:, ko, ts(tb, TB)],
                                    start=(ko == 0), stop=(ko == kd - 1))
                            nc.scalar.activation(
                                qT[:, m, ts(tb, TB)], ps[:], Ident,
                                bias=bias_sb[wname][:, m:m + 1])
                return qT

            def kv_proj(side, n):
                """Full k^T (f32r) and v (bf16) projections -> DRAM scratch."""
                xT = {1: x1T, 2: x2T}[side]
                kw, vw = f"k{side}", f"v{side}"
                with (
                    tc.tile_pool(name=f"wkv{side}", bufs=1) as wp,
                    tc.tile_pool(name=f"xs{side}", bufs=2) as xp,
                    tc.tile_pool(name=f"out{side}", bufs=3) as op,
                    tc.tile_pool(name=f"pskv{side}", bufs=2, space="PSUM") as pp,
                ):
                    wk_sb = wp.tile([P, kd, d], F32R, tag="wk")
                    wv_sb = wp.tile([P, kd, d], F32R, tag="wv")
                    nc.sync.dma_start(
                        wk_sb[:], W[kw].rearrange("(ko p) o -> p ko o", p=P).bitcast(F32R))
                    nc.sync.dma_start(
                        wv_sb[:], W[vw].rearrange("(ko p) o -> p ko o", p=P).bitcast(F32R))
                    xT3 = xT.rearrange("(ko p) t -> p ko t", p=P).bitcast(F32R)
                    for tt in range(n // TB):
                        x_sb = xp.tile([P, kd, TB], F32R, tag="x")
                        nc.sync.dma_start(x_sb[:], xT3[:, :, ts(tt, TB)])
                        # k^T: [dout-block, tokens]
                        for m in range(kd):
                            ps = pp.tile([P, TB], F32, tag="psk")
                            for ko in range(kd):
                                nc.tensor.matmul(
                                    ps[:], wk_sb[:, ko, ts(m, P)], x_sb[:, ko, :],
                                    start=(ko == 0), stop=(ko == kd - 1))
                            ko_sb = op.tile([P, TB], F32R, tag="ko")
                            nc.scalar.activation(ko_sb[:], ps[:], Ident,
                                                 bias=bias_sb[kw][:, m:m + 1])
                            nc.sync.dma_start(kT_d[side][ts(m, P), ts(tt, TB)], ko_sb[:])
                        # v natural: [tokens, dout], bias via K=1 ones matmul
                        for tsub in range(TB // P):
                            pslo = pp.tile([P, TB], F32, tag="pv0")
                            pshi = pp.tile([P, TB], F32, tag="pv1")
                            xblk = x_sb[:, :, ts(tsub, P)]
                            for ko in range(kd):
                                nc.tensor.matmul(pslo[:], xblk[:, ko], wv_sb[:, ko, 0:TB],
                                                 start=(ko == 0), stop=False)
                                nc.tensor.matmul(pshi[:], xblk[:, ko], wv_sb[:, ko, TB:d],
                                                 start=(ko == 0), stop=False)
                            nc.tensor.matmul(pslo[:], ones_sb[:], brow_sb[vw][:, 0:TB],
                                             start=False, stop=True)
                            nc.tensor.matmul(pshi[:], ones_sb[:], brow_sb[vw][:, TB:d],
                                             start=False, stop=True)
                            vo = op.tile([P, d], BF16, tag="vo")
                            nc.vector.tensor_copy(vo[:, 0:TB], pslo[:])
                            nc.vector.tensor_copy(vo[:, TB:d], pshi[:])
                            nc.sync.dma_start(v_d[side][tt * (TB // P) + tsub], vo[:])

            def pass_a(qT, kvside, pT_out, nb):
                """logits^T -> sigmoid -> probs^T (f32 out + bf16 scratch)."""
                nkv = kT_d[kvside].shape[1]
                kt3 = kT_d[kvside][:].rearrange("(ko p) t -> p ko t", p=P)
                nh = nb // TB
                with (
                    tc.tile_pool(name=f"ka{kvside}", bufs=3) as kp,
                    tc.tile_pool(name=f"pa{kvside}", bufs=3) as op,
                    tc.tile_pool(name=f"psa{kvside}", bufs=2, space="PSUM") as pp,
                ):
                    for j in range(nkv // P):
                        ktile = kp.tile([P, kd, P], F32R, tag="kt")
                        nc.sync.dma_start(ktile[:], kt3[:, :, ts(j, P)])
                        pss = [pp.tile([P, TB], F32, tag=f"ps{h}", name=f"ps{h}") for h in range(nh)]
                        for ko in range(kd):
                            for h in range(nh):
                                nc.tensor.matmul(
                                    pss[h][:], ktile[:, ko, :], qT[:, ko, ts(h, TB)],
                                    start=(ko == 0), stop=(ko == kd - 1))
                        pf = op.tile([P, nb], F32, tag="pf")
                        for h in range(nh):
                            nc.scalar.activation(pf[:, ts(h, TB)], pss[h][:],
                                                 Sigmoid, scale=scale)
                        pbt = op.tile([P, nb], BF16, tag="pb")
                        nc.vector.tensor_copy(pbt[:], pf[:])
                        nc.sync.dma_start(pT_out[ts(j, P), :], pf[:])
                        for m in range(nb // P):
                            nc.sync.dma_start(pb_d[kvside][m, j], pbt[:, ts(m, P)])

            def pass_b(kvside, ctx_out, nb):
                """ctx = probs @ v via probsT blocks as stationary."""
                nkv = v_d[kvside].shape[0] * P
                nj = nkv // P
                with (
                    tc.tile_pool(name=f"vb{kvside}", bufs=1) as vp,
                    tc.tile_pool(name=f"ptb{kvside}", bufs=4) as ptp,
                    tc.tile_pool(name=f"cb{kvside}", bufs=2) as cp,
                    tc.tile_pool(name=f"psb{kvside}", bufs=2, space="PSUM") as pp,
                ):
                    vsb = vp.tile([P, nj, d], BF16)
                    vd3 = v_d[kvside][:].rearrange("jo p e -> p jo e")
                    nchunk = 8
                    for c in range(nchunk):
                        cw = nj // nchunk
                        nc.sync.dma_start(vsb[:, ts(c, cw), :], vd3[:, ts(c, cw), :])
                    for m in range(nb // P):
                        pslo = pp.tile([P, TB], F32, tag="c0")
                        pshi = pp.tile([P, TB], F32, tag="c1")
                        for j in range(nj):
                            ptile = ptp.tile([P, P], BF16, tag="pt")
                            nc.sync.dma_start(ptile[:], pb_d[kvside][m, j])
                            nc.tensor.matmul(pslo[:], ptile[:], vsb[:, j, 0:TB],
                                             start=(j == 0), stop=(j == nj - 1))
                            nc.tensor.matmul(pshi[:], ptile[:], vsb[:, j, TB:d],
                                             start=(j == 0), stop=(j == nj - 1))
                        co = cp.tile([P, d], F32, tag="co")
                        nc.vector.tensor_copy(co[:, 0:TB], pslo[:])
                        nc.vector.tensor_copy(co[:, TB:d], pshi[:])
                        nc.sync.dma_start(ctx_out[ts(m, P), :], co[:])

            with tc.tile_pool(name="q2pool", bufs=1) as q2p:
                q2T = qT_proj(q2p, "q2", x2Tb, b2)
                with tc.tile_pool(name="q1pool", bufs=1) as q1p:
                    q1T = qT_proj(q1p, "q1", x1Tb, b1)
                    kv_proj(1, n1)
                    kv_proj(2, n2)
                    pass_a(q1T, 2, p2T, b1)   # probs2^T = sigmoid(k2 @ q1^T)
                pass_a(q2T, 1, p1T, b2)       # probs1^T = sigmoid(k1 @ q2^T)
            pass_b(2, ctx2, b1)
            pass_b(1, ctx1, b2)

    nc.compile()
    return nc


_CACHE = {}


def _get_nc(n1, n2, d):
    key = (n1, n2, d)
    if key not in _CACHE:
        _CACHE[key] = build(n1, n2, d)
    return _CACHE[key]


def make_in_maps(input_tensor1, input_tensor2, weights, ncores=NCORES):
    x1 = np.asarray(input_tensor1, dtype=np.float32)
    x2 = np.asarray(input_tensor2, dtype=np.float32)
    n1, d = x1.shape
    n2 = x2.shape[0]
    b1, b2 = n1 // ncores, n2 // ncores
    x1T = np.ascontiguousarray(x1.T)
    x2T = np.ascontiguousarray(x2.T)
    common = {"x1T": x1T, "x2T": x2T,
              "ones1": np.ones((1, P), np.float32)}
    for k, v in weights.items():
        common[k] = np.ascontiguousarray(np.asarray(v, dtype=np.float32))
    in_maps = []
    for c in range(ncores):
        m = dict(common)
        m["x1Tb"] = np.ascontiguousarray(x1T[:, c * b1:(c + 1) * b1])
        m["x2Tb"] = np.ascontiguousarray(x2T[:, c * b2:(c + 1) * b2])
        in_maps.append(m)
    return in_maps


def assemble(results, ncores=NCORES):
    ctx2 = np.concatenate([results[c]["ctx2"] for c in range(ncores)], axis=0)
    probs2 = np.concatenate(
        [np.ascontiguousarray(results[c]["p2T"].T) for c in range(ncores)], axis=0)
    ctx1 = np.concatenate([results[c]["ctx1"] for c in range(ncores)], axis=0)
    probs1 = np.concatenate(
        [np.ascontiguousarray(results[c]["p1T"].T) for c in range(ncores)], axis=0)
    return ctx2, probs2, ctx1, probs1


def kernel(input_tensor1, input_tensor2,
           Wq1, bq1, Wk1, bk1, Wv1, bv1,
           Wq2, bq2, Wk2, bk2, Wv2, bv2):
    x1 = np.asarray(input_tensor1, dtype=np.float32)
    x2 = np.asarray(input_tensor2, dtype=np.float32)
    n1, d = x1.shape
    n2 = x2.shape[0]
    weights = {"Wq1": Wq1, "bq1": bq1, "Wk1": Wk1, "bk1": bk1,
               "Wv1": Wv1, "bv1": bv1, "Wq2": Wq2, "bq2": bq2,
               "Wk2": Wk2, "bk2": bk2, "Wv2": Wv2, "bv2": bv2}
    nc = _get_nc(n1, n2, d)
    in_maps = make_in_maps(x1, x2, weights)
    res = run_bass_kernel_spmd(nc, in_maps, list(range(NCORES)))
    return assemble(res.results)
